# revision 26
# baseline (speedup 1.0000x reference)
"""AeroForceLoss Trainium2 kernel.

Computes, over prediction/target [N,4], normals [N,3], areas [N,1] with
N = B*S (B=16 segments of S=131072 points):

    diff = prediction - target
    base_loss = mean(diff^2)
    vec = (diff[...,0:1] + diff[...,1:4]) * normals * areas   (per segment)
    d_fa[b] = sum_s vec                                       [B,3]
    force_loss = mean_b ||d_fa[b]||_2
    out = base_loss + 0.1 * force_loss

Data-parallel across 8 NeuronCores: core i handles segments 2i, 2i+1.

Per-core dataflow (per chunk of points):
  DVE:  diff = pred - targ;  s = d0 + dK;  w = s * m (in place)
  POOL: m = normals * area (area broadcast over the 3 components)
  ACT:  base-loss Square+accum (in place on diff), + DMA issue ring 2
  PE:   ones-matmul partition-reduce of w into a per-segment PSUM row
        [1, 384] (<=512-wide slices, columns folded mod 384; 384 % 3 == 0
        preserves the xyz interleave), accumulated across chunks
  Host: gathers the 8 tiny accumulator tiles and does the final scalar
        math (sum-of-squares, per-segment force norms, means).
"""

import numpy as np

import concourse.bacc as bacc
import concourse.mybir as mybir
import concourse.tile as tile
from concourse import bass_utils

FORCE_LOSS_WEIGHT = 0.1

B = 16  # segments (batch)
S = 131072  # points per segment
N_CORES = 8
SEGS = B // N_CORES  # segments per core = 2
P = 128  # SBUF partitions
R = 512  # PSUM force-accumulator columns per (segment, K) plane

F32 = mybir.dt.float32


def chunk_plan(q):
    """Chunk sizes (points per partition) within one segment of q ppp,
    ramping small -> large. The caller reverses it for the last segment
    so both the pipeline fill chunk and the tail chunk are small."""
    if q < 16:
        return [q // 2, q - q // 2]
    e = q // 8
    return [e, 3 * e, q - 4 * e]


def build_nc(segs=SEGS, s_len=S):
    """Build the per-core SPMD Bass module.

    Inputs (per core): pred/targ [segs*s_len, 4], nrm [segs*s_len, 3],
    area [segs*s_len, 1], all f32.
    Outputs: acc_out [128, ntot] (per-chunk per-partition sum(diff^2)),
    force_out [1, segs*R] (per-segment folded force column sums; host
    reduces col c of segment s into component c % 3).
    """
    q = s_len // P  # points per partition per segment
    assert s_len % P == 0
    plan = chunk_plan(q)
    assert sum(plan) == q
    nch = len(plan)
    ntot = segs * nch
    chmax = max(plan)
    r = R  # psum cols per (segment, K): one full bank, so the three
    # K-planes land in distinct banks (distinct accumulation groups)
    assert chmax <= r

    nc = bacc.Bacc("TRN2", target_bir_lowering=False, debug=False)
    npts = segs * s_len
    pred = nc.dram_tensor("pred", [npts, 4], F32, kind="ExternalInput").ap()
    targ = nc.dram_tensor("targ", [npts, 4], F32, kind="ExternalInput").ap()
    nrm = nc.dram_tensor("nrm", [npts, 3], F32, kind="ExternalInput").ap()
    area = nc.dram_tensor("area", [npts, 1], F32, kind="ExternalInput").ap()
    acc_dram = nc.dram_tensor("acc_out", [P, ntot], F32, kind="ExternalOutput").ap()
    force_dram = nc.dram_tensor(
        "force_out", [1, segs * 3 * chmax], F32, kind="ExternalOutput"
    ).ap()

    # Partition p of segment s holds points [s*s_len + p*q, ... + q).
    pred_v = pred.rearrange("(s p j) c -> s p (j c)", s=segs, p=P)
    targ_v = targ.rearrange("(s p j) c -> s p (j c)", s=segs, p=P)
    nrm_v = nrm.rearrange("(s p j) c -> s p (j c)", s=segs, p=P)
    area_v = area.rearrange("(s p j) c -> s p (j c)", s=segs, p=P)

    with tile.TileContext(nc) as tc:
        with (
            tc.tile_pool(name="loads", bufs=5) as loads,
            tc.tile_pool(name="work", bufs=3) as work,
            tc.tile_pool(name="accp", bufs=1) as accp,
            tc.tile_pool(name="psp", bufs=1, space="PSUM") as psp,
        ):
            acc = accp.tile([P, ntot], F32)
            ones = accp.tile([P, 1], F32)
            nc.vector.memset(ones, 1.0)
            forces = accp.tile([1, segs * 3 * chmax], F32)

            # Flat chunk schedule. Small fill chunk first; reversed on the
            # last segment so the serial-dependency tail chunk is small too.
            sched = []
            for s in range(segs):
                seg_plan = plan if s + 1 < segs else plan[::-1]
                off = 0
                for k, ch in enumerate(seg_plan):
                    sched.append((s, k, ch, off))
                    off += ch
            pstiles = {
                s: psp.tile([1, 3 * r], F32, name=f"ps{s}", tag=f"ps{s}")
                for s in range(segs)
            }
            inflight = {}

            def emit_loads(idx):
                s, k, ch, o = sched[idx]
                tp = loads.tile([P, chmax * 4], F32, tag="tp", name="tp")[:, :ch * 4]
                tt = loads.tile([P, chmax * 4], F32, tag="tt", name="tt")[:, :ch * 4]
                tn = loads.tile([P, chmax * 3], F32, tag="tn", name="tn")[:, :ch * 3]
                ta = loads.tile([P, chmax], F32, tag="ta", name="ta")[:, :ch]
                # Two HWDGE rings: SP gets pred+area, ACT gets targ+nrm.
                # The ACT issues are emitted LAG chunks ahead of the ACT
                # squares, so a stalled square never starves the ring.
                nc.sync.dma_start(out=tp, in_=pred_v[s, :, o * 4:(o + ch) * 4])
                nc.scalar.dma_start(out=tt, in_=targ_v[s, :, o * 4:(o + ch) * 4])
                nc.scalar.dma_start(out=tn, in_=nrm_v[s, :, o * 3:(o + ch) * 3])
                nc.sync.dma_start(out=ta, in_=area_v[s, :, o:o + ch])
                inflight[idx] = (tp, tt, tn, ta)

            def emit_compute(idx):
                s, k, ch, o = sched[idx]
                g = s * nch + k
                ps = pstiles[s]
                tp, tt, tn, ta = inflight.pop(idx)

                # diff = pred - targ  (DVE)
                td = work.tile([P, chmax * 4], F32, tag="td", name="td")[:, :ch * 4]
                nc.vector.tensor_sub(td, tp, tt)

                # m[K*ch + j] = normals[j,K] * areas[j], K-major planar
                # (POOL; broadcast area over the 3 planes)
                tm = work.tile([P, chmax * 3], F32, tag="tm", name="tm")[:, :ch * 3]
                tm_kj = tm.rearrange("p (c j) -> p c j", c=3)
                tn_kj = tn.rearrange("p (j c) -> p c j", c=3)
                ta_b = ta.unsqueeze(1).broadcast_to((P, 3, ch))
                nc.gpsimd.tensor_mul(tm_kj, tn_kj, ta_b)

                # s[K*ch + j] = d0[j] + dK[j], K-major planar  (DVE)
                td4 = td.rearrange("p (j c) -> p c j", c=4)
                d0b = td4[:, 0:1, :].broadcast_to((P, 3, ch))
                ts = work.tile([P, chmax * 3], F32, tag="ts", name="ts")[:, :ch * 3]
                ts_kj = ts.rearrange("p (c j) -> p c j", c=3)
                nc.vector.tensor_add(ts_kj, d0b, td4[:, 1:4, :])

                # base loss: sum(diff^2) into acc[:, g]  (ACT, fused,
                # in-place square after s has consumed td)
                nc.scalar.activation(
                    out=td,
                    in_=td,
                    func=mybir.ActivationFunctionType.Square,
                    accum_out=acc[:, g:g + 1],
                )

                # w = s * m, dense planar, in place into ts  (DVE)
                nc.vector.tensor_mul(ts, ts, tm)

                # PE: per-K partition-reduce of w into psum plane K.
                for K in range(3):
                    nc.tensor.matmul(
                        ps[:, K * r:K * r + ch],
                        ones,
                        ts[:, K * ch:(K + 1) * ch],
                        start=(k == 0),
                        stop=(k == nch - 1),
                    )
                if k == nch - 1:
                    # copy the finished segment accumulator out of PSUM (ACT):
                    # only the written [0, chmax) prefix of each K-plane bank.
                    ps_used = ps.rearrange("o (c r) -> o c r", c=3)[:, :, :chmax]
                    fdst = forces[:, s * 3 * chmax:(s + 1) * 3 * chmax]
                    nc.scalar.copy(fdst.rearrange("o (c j) -> o c j", c=3), ps_used)

            lag = min(3, len(sched))
            for idx in range(lag):
                emit_loads(idx)
            for idx in range(len(sched)):
                emit_compute(idx)
                if idx + lag < len(sched):
                    emit_loads(idx + lag)
            nc.sync.dma_start(out=acc_dram, in_=acc)
            nc.sync.dma_start(out=force_dram, in_=forces)
    nc.compile()
    return nc


_NC_CACHE = {}


def _get_nc():
    if "nc" not in _NC_CACHE:
        _NC_CACHE["nc"] = build_nc()
    return _NC_CACHE["nc"]


def combine_host(accs, forces, segs=SEGS, s_len=S):
    """accs: [n_cores, 128, ntot], forces: [n_cores, 3, segs*r]
    -> scalar loss (float64 math)."""
    n_cores = accs.shape[0]
    ss = accs.sum(dtype=np.float64)
    base = ss / (n_cores * segs * s_len * 4)
    r = forces.shape[-1] // (segs * 3)
    # forces[core, 0, s*3r + K*r + j] -> F[core, s, K]
    f = forces.reshape(n_cores, segs, 3, r).sum(axis=3, dtype=np.float64)
    norms = np.sqrt((f * f).sum(axis=-1))  # [n_cores, segs]
    force = norms.mean()
    return base + FORCE_LOSS_WEIGHT * force


def kernel(prediction, target, normals, areas, batch_size=B, sim_len=S, **_):
    assert int(batch_size) == B and int(sim_len) == S
    prediction = np.ascontiguousarray(np.asarray(prediction, dtype=np.float32))
    target = np.ascontiguousarray(np.asarray(target, dtype=np.float32))
    normals = np.ascontiguousarray(np.asarray(normals, dtype=np.float32))
    areas = np.ascontiguousarray(np.asarray(areas, dtype=np.float32))

    nc = _get_nc()
    rows = SEGS * S
    in_maps = [
        {
            "pred": prediction[i * rows:(i + 1) * rows],
            "targ": target[i * rows:(i + 1) * rows],
            "nrm": normals[i * rows:(i + 1) * rows],
            "area": areas[i * rows:(i + 1) * rows],
        }
        for i in range(N_CORES)
    ]
    res = bass_utils.run_bass_kernel_spmd(nc, in_maps, core_ids=list(range(N_CORES)))
    accs = np.stack([r["acc_out"] for r in res.results])
    forces = np.stack([r["force_out"] for r in res.results])
    return np.float32(combine_host(accs, forces))


# revision 27
# speedup vs baseline: 1.3348x; 1.3348x over previous
"""AeroForceLoss Trainium2 kernel.

Computes, over prediction/target [N,4], normals [N,3], areas [N,1] with
N = B*S (B=16 segments of S=131072 points):

    diff = prediction - target
    base_loss = mean(diff^2)
    vec = (diff[...,0:1] + diff[...,1:4]) * normals * areas   (per segment)
    d_fa[b] = sum_s vec                                       [B,3]
    force_loss = mean_b ||d_fa[b]||_2
    out = base_loss + 0.1 * force_loss

Data-parallel across 8 NeuronCores: core i handles segments 2i, 2i+1.

Per-core dataflow, chunked [128 partitions x ch points]:
  DVE:  diff = pred - targ;  m = normals*area;  s = d0 + dK;
        per-K fused multiply+reduce (scalar_tensor_tensor accum)
  ACT:  base-loss Square+accum into per-chunk columns, + DMA ring 2
  Host: gathers the 8 per-core [128, 16] accumulators and finishes
        (sum-of-squares, per-segment force norms, means).

DMA: two HWDGE rings (SP: pred+area, ACT: targ+nrm), with load issues
emitted LAG chunks ahead of compute so a stalled ACT square never
starves its ring.
"""

import numpy as np

import concourse.bacc as bacc
import concourse.mybir as mybir
import concourse.tile as tile
from concourse import bass_utils

FORCE_LOSS_WEIGHT = 0.1

B = 16  # segments (batch)
S = 131072  # points per segment
N_CORES = 8
SEGS = B // N_CORES  # segments per core = 2
P = 128  # SBUF partitions
CH = 512  # points per partition per chunk

F32 = mybir.dt.float32


def build_nc(segs=SEGS, s_len=S, ch=CH, lag=2, loads_bufs=3, work_bufs=2):
    """Build the per-core SPMD Bass module.

    Inputs (per core): pred/targ [segs*s_len, 4], nrm [segs*s_len, 3],
    area [segs*s_len, 1], all f32. Output: acc_out [128, 4*ntot]:
    col g in [0, ntot) = per-partition sum(diff^2) of chunk g;
    col ntot + 3g + K = per-partition sum((d0+dK)*nK*a) of chunk g.
    """
    q = s_len // P  # points per partition per segment
    assert s_len % P == 0 and q % ch == 0
    nch = q // ch
    ntot = segs * nch

    nc = bacc.Bacc("TRN2", target_bir_lowering=False, debug=False)
    npts = segs * s_len
    pred = nc.dram_tensor("pred", [npts, 4], F32, kind="ExternalInput").ap()
    targ = nc.dram_tensor("targ", [npts, 4], F32, kind="ExternalInput").ap()
    nrm = nc.dram_tensor("nrm", [npts, 3], F32, kind="ExternalInput").ap()
    area = nc.dram_tensor("area", [npts, 1], F32, kind="ExternalInput").ap()
    acc_dram = nc.dram_tensor("acc_out", [P, 4 * ntot], F32, kind="ExternalOutput").ap()

    # Partition p of segment s holds points [s*s_len + p*q, ... + q).
    pred_v = pred.rearrange("(s p j) c -> s p (j c)", s=segs, p=P)
    targ_v = targ.rearrange("(s p j) c -> s p (j c)", s=segs, p=P)
    nrm_v = nrm.rearrange("(s p j) c -> s p (j c)", s=segs, p=P)
    area_v = area.rearrange("(s p j) c -> s p (j c)", s=segs, p=P)

    sched = [(s, k) for s in range(segs) for k in range(nch)]
    inflight = {}

    with tile.TileContext(nc) as tc:
        with (
            tc.tile_pool(name="loads", bufs=loads_bufs) as loads,
            tc.tile_pool(name="work", bufs=work_bufs) as work,
            tc.tile_pool(name="accp", bufs=1) as accp,
        ):
            acc = accp.tile([P, 4 * ntot], F32)

            def emit_loads(idx):
                s, k = sched[idx]
                tp = loads.tile([P, ch * 4], F32, tag="tp", name="tp")
                tt = loads.tile([P, ch * 4], F32, tag="tt", name="tt")
                tn = loads.tile([P, ch * 3], F32, tag="tn", name="tn")
                ta = loads.tile([P, ch], F32, tag="ta", name="ta")
                nc.sync.dma_start(out=tp, in_=pred_v[s, :, k * ch * 4:(k + 1) * ch * 4])
                nc.scalar.dma_start(out=tt, in_=targ_v[s, :, k * ch * 4:(k + 1) * ch * 4])
                nc.scalar.dma_start(out=tn, in_=nrm_v[s, :, k * ch * 3:(k + 1) * ch * 3])
                nc.sync.dma_start(out=ta, in_=area_v[s, :, k * ch:(k + 1) * ch])
                inflight[idx] = (tp, tt, tn, ta)

            def emit_compute(idx):
                s, k = sched[idx]
                g = s * nch + k
                tp, tt, tn, ta = inflight.pop(idx)

                # diff = pred - targ  (DVE)
                td = work.tile([P, ch * 4], F32, tag="td", name="td")
                nc.vector.tensor_sub(td, tp, tt)

                # base loss: sum(diff^2) into acc[:, g]  (ACT, fused)
                tsq = work.tile([P, ch * 4], F32, tag="tsq", name="tsq")
                nc.scalar.activation(
                    out=tsq,
                    in_=td,
                    func=mybir.ActivationFunctionType.Square,
                    accum_out=acc[:, g:g + 1],
                )

                # m = normals * areas (broadcast area over 3 comps)  (DVE)
                tn3 = tn.rearrange("p (j c) -> p j c", c=3)
                ta_b = ta.unsqueeze(2).broadcast_to((P, ch, 3))
                tm = work.tile([P, ch * 3], F32, tag="tm", name="tm")
                tm3 = tm.rearrange("p (j c) -> p j c", c=3)
                nc.vector.tensor_mul(tm3, tn3, ta_b)

                # sK = d0 + dK  (DVE, d0 broadcast over K)
                td4 = td.rearrange("p (j c) -> p j c", c=4)
                d0b = td4[:, :, 0:1].broadcast_to((P, ch, 3))
                ts = work.tile([P, ch * 3], F32, tag="ts", name="ts")
                ts3 = ts.rearrange("p (j c) -> p j c", c=3)
                nc.vector.tensor_add(ts3, d0b, td4[:, :, 1:4])

                # per K: acc[:, ntot+3g+K] = sum(sK*mK)  (DVE, fused)
                tw = work.tile([P, ch * 3], F32, tag="tw", name="tw")
                tw3 = tw.rearrange("p (j c) -> p j c", c=3)
                for K in range(3):
                    col = ntot + 3 * g + K
                    nc.vector.scalar_tensor_tensor(
                        out=tw3[:, :, K:K + 1],
                        in0=ts3[:, :, K:K + 1],
                        scalar=0.0,
                        in1=tm3[:, :, K:K + 1],
                        op0=mybir.AluOpType.add,
                        op1=mybir.AluOpType.mult,
                        accum_out=acc[:, col:col + 1],
                    )

            lag = min(lag, len(sched))
            for idx in range(lag):
                emit_loads(idx)
            for idx in range(len(sched)):
                emit_compute(idx)
                if idx + lag < len(sched):
                    emit_loads(idx + lag)
            nc.sync.dma_start(out=acc_dram, in_=acc)
    nc.compile()
    return nc


_NC_CACHE = {}


def _get_nc():
    if "nc" not in _NC_CACHE:
        _NC_CACHE["nc"] = build_nc()
    return _NC_CACHE["nc"]


def combine_host(accs, segs=SEGS, s_len=S, ch=CH):
    """accs: [n_cores, 128, 4*ntot] -> scalar loss (float64 math)."""
    n_cores = accs.shape[0]
    q = s_len // P
    nch = q // ch
    ntot = segs * nch
    ss = accs[:, :, :ntot].sum(dtype=np.float64)
    base = ss / (n_cores * segs * s_len * 4)
    f = accs[:, :, ntot:].reshape(n_cores, P, segs, nch, 3)
    f = f.sum(axis=(1, 3), dtype=np.float64)  # [n_cores, segs, 3]
    norms = np.sqrt((f * f).sum(axis=-1))  # [n_cores, segs]
    force = norms.mean()
    return base + FORCE_LOSS_WEIGHT * force


def kernel(prediction, target, normals, areas, batch_size=B, sim_len=S, **_):
    assert int(batch_size) == B and int(sim_len) == S
    prediction = np.ascontiguousarray(np.asarray(prediction, dtype=np.float32))
    target = np.ascontiguousarray(np.asarray(target, dtype=np.float32))
    normals = np.ascontiguousarray(np.asarray(normals, dtype=np.float32))
    areas = np.ascontiguousarray(np.asarray(areas, dtype=np.float32))

    nc = _get_nc()
    rows = SEGS * S
    in_maps = [
        {
            "pred": prediction[i * rows:(i + 1) * rows],
            "targ": target[i * rows:(i + 1) * rows],
            "nrm": normals[i * rows:(i + 1) * rows],
            "area": areas[i * rows:(i + 1) * rows],
        }
        for i in range(N_CORES)
    ]
    res = bass_utils.run_bass_kernel_spmd(nc, in_maps, core_ids=list(range(N_CORES)))
    accs = np.stack([r["acc_out"] for r in res.results])
    return np.float32(combine_host(accs))
